# revision 1
# baseline (speedup 1.0000x reference)
"""Trainium2 Bass kernel for adaptive_high_order_residual_v2 (ORDER=2 masked
sign-binarization, per-row stats).

Full-input contract: kernel(x, mask) takes the complete (4096, 11008) arrays,
shards rows across 8 NeuronCores (512 rows each; per-row reductions make this
embarrassingly parallel), runs one SPMD Bass program, and concatenates the
per-core outputs.

Math per row (ORDER = 2, exact restructuring of the reference):
  t    = x * m                      (masked input)
  mean1 = sum(t)/cnt ; var1 = sum(t^2)/cnt - mean1^2 ; s1 = sqrt(var1 * 2/pi)
  b1   = sign(x - mean1)            (valid entries only; invalid masked later)
  q    = (|x - mean1| - s1) * b1*m  (== residual_2 of the reference: d - s1*b1)
  mean2 = sum(q)/cnt ; var2 = sum(q^2)/cnt - mean2^2 ; s2 = sqrt(var2 * 2/pi)
  b2   = sign(q - mean2)
  out  = ((mean1 + mean2) + s1*b1 + s2*b2) * m

Engine split per 128x2752 chunk:
  ACT: mask cast (+cnt accum), square(T) (+r2), Sign->b1, Abs->|d|,
       square(q) (+sum q^2), Sign->b2
  DVE: T=x*m (+r1 accum), b1m=b1*m16 (bf16 2x), q=(|d|-s1)*b1m (+sum q),
       p1=s1*b1m+K (2x), p2=s2*b2+p1, out=p2*m
"""

import sys

import numpy as np

sys.path.insert(0, "/opt/trn_rl_repo")

R = 512          # rows per core
N = 11008        # columns
P = 128          # SBUF partitions per row-block
NBLK = R // P    # 4 blocks per core
CW = 2752        # column chunk width
NCH = N // CW    # 4 chunks per block
NCORES = 8
C2 = 0.6366197723675814  # 2/pi

_CACHE = {}


def _build_program():
    import concourse.bacc as bacc
    import concourse.mybir as mybir
    from concourse.tile import TileContext

    F32 = mybir.dt.float32
    BF16 = mybir.dt.bfloat16
    U8 = mybir.dt.uint8
    Alu = mybir.AluOpType
    Act = mybir.ActivationFunctionType

    nc = bacc.Bacc()
    x = nc.dram_tensor("x", [R, N], F32, kind="ExternalInput")
    mk = nc.dram_tensor("mask", [R, N], U8, kind="ExternalInput")
    out = nc.dram_tensor("out", [R, N], F32, kind="ExternalOutput")

    with TileContext(nc) as tc:
        with (
            tc.tile_pool(name="xq", bufs=6) as xq_pool,    # x tile, later holds q
            tc.tile_pool(name="m8", bufs=2) as m8_pool,    # raw u8 mask (cast only)
            tc.tile_pool(name="m16", bufs=5) as m16_pool,  # bf16 mask
            tc.tile_pool(name="bmp", bufs=5) as bm_pool,   # masked sign1 (bf16)
            tc.tile_pool(name="b2p", bufs=1) as b2_pool,   # sign2 + ACT garbage
            tc.tile_pool(name="w", bufs=4) as w_pool,      # f32 rotating work
            tc.tile_pool(name="tp", bufs=3) as t_pool,     # T tiles (stage 1)
            tc.tile_pool(name="sc", bufs=2) as sc_pool,    # scalars + accums
        ):
            for b in range(NBLK):
                r0 = b * P

                xt = [
                    xq_pool.tile([P, CW], F32, name=f"xt{b}_{c}", tag="xq")
                    for c in range(NCH)
                ]
                mt = [
                    m8_pool.tile([P, CW], U8, name=f"mt{b}_{c}", tag="m8")
                    for c in range(NCH)
                ]
                m16 = [
                    m16_pool.tile([P, CW], BF16, name=f"m16_{b}_{c}", tag="m16")
                    for c in range(NCH)
                ]
                bm = [
                    bm_pool.tile([P, CW], BF16, name=f"bm{b}_{c}", tag="bm")
                    for c in range(NCH)
                ]
                # accumulators, chunk-major interleave: col = c*nq + q so the
                # pairwise tree reduce uses contiguous 2-D slices
                acc1 = sc_pool.tile([P, 3 * 2 * NCH], F32, name=f"acc1_{b}", tag="acc1")
                acc2 = sc_pool.tile([P, 2 * (NCH + 1)], F32, name=f"acc2_{b}", tag="acc2")
                st1 = sc_pool.tile([P, 3], F32, name=f"st1_{b}", tag="st1")
                st2 = sc_pool.tile([P, 2], F32, name=f"st2_{b}", tag="st2")
                red1 = sc_pool.tile([P, 18], F32, name=f"red1_{b}", tag="red1")
                red2 = sc_pool.tile([P, 6], F32, name=f"red2_{b}", tag="red2")
                sv = sc_pool.tile([P, 24], F32, name=f"sv_{b}", tag="sv")

                def col(t, i):
                    return t[:, i : i + 1]

                # piece lists: (chunk, col offset, width). The first chunk of
                # each stage is split in half so the consuming engine starts
                # ~1.2us sooner after a stage boundary; block 0 also splits
                # the very first DMA for a faster ramp.
                H = CW // 2

                def pieces_for(split_first, split_last=False):
                    ps = []
                    for c in range(NCH):
                        if (c == 0 and split_first) or (
                            c == NCH - 1 and split_last
                        ):
                            ps.append((c, 0, H))
                            ps.append((c, H, H))
                        else:
                            ps.append((c, 0, CW))
                    return ps

                s1_pieces = pieces_for(False)
                s2_pieces = pieces_for(False)
                s3_pieces = pieces_for(False, split_last=(b == NBLK - 1))

                def reduce_cols(dst, acc, nq, npieces, red):
                    # sum piece-major accum columns: col = p*nq + q
                    if npieces == 8:
                        nc.vector.tensor_add(
                            red[:, 0 : 4 * nq], acc[:, 0 : 4 * nq],
                            acc[:, 4 * nq : 8 * nq],
                        )
                        nc.vector.tensor_add(
                            red[:, 4 * nq : 6 * nq], red[:, 0 : 2 * nq],
                            red[:, 2 * nq : 4 * nq],
                        )
                        nc.vector.tensor_add(
                            dst, red[:, 4 * nq : 5 * nq], red[:, 5 * nq : 6 * nq]
                        )
                    elif npieces == 4:
                        nc.vector.tensor_add(
                            red[:, 0 : 2 * nq], acc[:, 0 : 2 * nq],
                            acc[:, 2 * nq : 4 * nq],
                        )
                        nc.vector.tensor_add(dst, red[:, 0:nq], red[:, nq : 2 * nq])
                    elif npieces == 5:
                        nc.vector.tensor_add(
                            red[:, 0 : 2 * nq], acc[:, 0 : 2 * nq],
                            acc[:, 2 * nq : 4 * nq],
                        )
                        nc.vector.tensor_add(
                            red[:, 2 * nq : 3 * nq], red[:, 0:nq], red[:, nq : 2 * nq]
                        )
                        nc.vector.tensor_add(
                            dst, red[:, 2 * nq : 3 * nq], acc[:, 4 * nq : 5 * nq]
                        )
                    else:
                        raise AssertionError(npieces)

                # ------------- stage 1: masked first-order stats -------------
                for i, (c, o, wd) in enumerate(s1_pieces):
                    if o == 0:
                        nc.sync.dma_start(
                            xt[c][:, 0:wd], x[r0 : r0 + P, c * CW : c * CW + wd]
                        )
                        nc.sync.dma_start(
                            mt[c][:, 0:wd], mk[r0 : r0 + P, c * CW : c * CW + wd]
                        )
                    else:
                        nc.sync.dma_start(
                            xt[c][:, o : o + wd],
                            x[r0 : r0 + P, c * CW + o : c * CW + o + wd],
                        )
                        nc.sync.dma_start(
                            mt[c][:, o : o + wd],
                            mk[r0 : r0 + P, c * CW + o : c * CW + o + wd],
                        )
                    # mask cast to bf16 + cnt partial
                    nc.scalar.activation(
                        m16[c][:, o : o + wd],
                        mt[c][:, o : o + wd],
                        Act.Copy,
                        accum_out=col(acc1, i * 3 + 0),
                    )
                    # T = x*m + r1 partial
                    tt = t_pool.tile([P, wd], F32, name=f"tt{b}_{i}", tag="tp")
                    nc.vector.scalar_tensor_tensor(
                        tt[:],
                        xt[c][:, o : o + wd],
                        1.0,
                        mt[c][:, o : o + wd],
                        Alu.bypass,
                        Alu.mult,
                        accum_out=col(acc1, i * 3 + 1),
                    )
                    # r2 partial: sum(T^2); output value unused -> dump it
                    # into the bm tile (Sign1 overwrites it in stage 2)
                    nc.scalar.activation(
                        bm[c][:, o : o + wd],
                        tt[:],
                        Act.Square,
                        accum_out=col(acc1, i * 3 + 2),
                    )

                reduce_cols(st1[:], acc1[:], 3, len(s1_pieces), red1)
                cnt, r1, r2 = col(st1, 0), col(st1, 1), col(st1, 2)
                cntc, inv = col(sv, 0), col(sv, 1)
                mean1, e1, nm1 = col(sv, 2), col(sv, 3), col(sv, 4)
                nv1, v1c, s1 = col(sv, 5), col(sv, 6), col(sv, 7)
                tA, tB, tC, tD = col(sv, 16), col(sv, 17), col(sv, 18), col(sv, 19)
                tE, tF = col(sv, 20), col(sv, 21)

                def newton_sqrt(dst, seed, vsq, t1, t2, mid, vh):
                    # dst = sqrt(vsq), one Newton step from the ACT seed (HW
                    # Sqrt is ~7e-6 rel; one step lands ~2e-11).
                    # TT/TS only (the STT ISA struct allows one sync wait).
                    nc.vector.tensor_scalar(vh[:], vsq[:], 0.5, None, Alu.mult)
                    nc.vector.reciprocal(t1[:], seed[:])
                    nc.vector.tensor_mul(t2[:], vh[:], t1[:])
                    nc.vector.tensor_scalar(t1[:], seed[:], 0.5, None, Alu.mult)
                    nc.vector.tensor_add(dst, t1[:], t2[:])

                nc.vector.tensor_scalar(cntc, cnt, 1.0, None, Alu.max)
                nc.vector.reciprocal(inv, cntc)
                nc.vector.tensor_mul(mean1, r1, inv)
                nc.vector.tensor_scalar(nm1, mean1, -1.0, None, Alu.mult)
                nc.vector.tensor_mul(e1, r2, inv)
                nc.vector.tensor_mul(nv1, mean1, mean1)
                nc.vector.tensor_sub(tE, e1, nv1)
                nc.vector.tensor_scalar(v1c, tE, C2, 1e-30, Alu.mult, Alu.max)
                nc.scalar.activation(tC, v1c, Act.Sqrt)
                newton_sqrt(s1, tC, v1c, tA, tB, tD, tF)

                # ------------- stage 2: residual q + second-order stats ------
                for i, (c, o, wd) in enumerate(s2_pieces):
                    xs = xt[c][:, o : o + wd]
                    ms = m16[c][:, o : o + wd]
                    bs = bm[c][:, o : o + wd]
                    # sign1 straight into the bm tile, then mask in place
                    nc.scalar.activation(bs, xs, Act.Sign, bias=nm1)
                    ab = w_pool.tile([P, wd], F32, name=f"ab{b}_{i}", tag="w")
                    nc.scalar.activation(ab[:], xs, Act.Abs, bias=nm1)
                    # masked sign1 (bf16 2x, in place)
                    nc.vector.tensor_mul(bs, bs, ms)
                    # q = (|d| - s1) * b1m, overwrites the x tile; accum sum(q)
                    nc.vector.scalar_tensor_tensor(
                        xs,
                        ab[:],
                        s1,
                        bs,
                        Alu.subtract,
                        Alu.mult,
                        accum_out=col(acc2, i * 2 + 0),
                    )
                    nc.scalar.activation(
                        ab[:], xs, Act.Square, accum_out=col(acc2, i * 2 + 1)
                    )

                reduce_cols(st2[:], acc2[:], 2, len(s2_pieces), red2)
                sq, sq2 = col(st2, 0), col(st2, 1)
                mean2, e2, nm2 = col(sv, 9), col(sv, 10), col(sv, 11)
                nv2, v2c, s2, kk = col(sv, 12), col(sv, 13), col(sv, 14), col(sv, 15)

                nc.vector.tensor_mul(mean2, sq, inv)
                nc.vector.tensor_scalar(nm2, mean2, -1.0, None, Alu.mult)
                nc.vector.tensor_mul(e2, sq2, inv)
                nc.vector.tensor_mul(nv2, mean2, mean2)
                nc.vector.tensor_sub(tE, e2, nv2)
                nc.vector.tensor_scalar(v2c, tE, C2, 1e-30, Alu.mult, Alu.max)
                nc.scalar.activation(tC, v2c, Act.Sqrt)
                newton_sqrt(s2, tC, v2c, tA, tB, tD, tF)
                nc.vector.tensor_add(kk, mean1, mean2)

                # ------------- stage 3: output assembly ----------------------
                for i, (c, o, wd) in enumerate(s3_pieces):
                    qs = xt[c][:, o : o + wd]
                    ms = m16[c][:, o : o + wd]
                    bs = bm[c][:, o : o + wd]
                    b2t = b2_pool.tile([P, wd], BF16, name=f"b2_{b}_{i}", tag="b2")
                    nc.scalar.activation(b2t[:], qs, Act.Sign, bias=nm2)
                    p1 = w_pool.tile([P, wd], F32, name=f"p1_{b}_{i}", tag="w")
                    # p1 = s1*b1m + K  (TS dual-scalar, 2x)
                    nc.vector.tensor_scalar(p1[:], bs, s1, kk, Alu.mult, Alu.add)
                    # p1 += s2*b2, then *= m  (in-place, one work tile/chunk)
                    nc.vector.scalar_tensor_tensor(
                        p1[:], b2t[:], s2, p1[:], Alu.mult, Alu.add
                    )
                    nc.vector.tensor_mul(p1[:], p1[:], ms)
                    nc.sync.dma_start(
                        out[r0 : r0 + P, c * CW + o : c * CW + o + wd], p1[:]
                    )

    return nc


def get_program():
    if "nc" not in _CACHE:
        nc = _build_program()
        # Bacc defers register allocation etc. to compile()/finalize();
        # the spmd exec path serializes without finalizing.
        nc.finalize()
        _CACHE["nc"] = nc
    return _CACHE["nc"]


def kernel(x: np.ndarray, mask: np.ndarray) -> np.ndarray:
    import time

    from concourse.bass_utils import run_bass_kernel_spmd

    x = np.ascontiguousarray(np.asarray(x, dtype=np.float32))
    mask = np.ascontiguousarray(np.asarray(mask))
    if mask.dtype == np.bool_ or mask.dtype == np.uint8:
        mask_u8 = mask.view(np.uint8)
    else:
        mask_u8 = (mask != 0).astype(np.uint8)
    assert x.shape == (R * NCORES, N), x.shape
    assert mask_u8.shape == (R * NCORES, N), mask_u8.shape

    nc = get_program()
    in_maps = [
        {
            "x": x[k * R : (k + 1) * R],
            "mask": mask_u8[k * R : (k + 1) * R],
        }
        for k in range(NCORES)
    ]
    last_err = None
    for attempt in range(3):
        try:
            res = run_bass_kernel_spmd(nc, in_maps, core_ids=list(range(NCORES)))
            return np.concatenate([r["out"] for r in res.results], axis=0)
        except Exception as e:  # transient NRT/device hiccups
            last_err = e
            if attempt < 2:
                time.sleep(10)
    raise last_err


if __name__ == "__main__":
    xs = np.random.randn(R * NCORES, N).astype(np.float32)
    ms = (np.random.randint(0, 2, (R * NCORES, N))).astype(bool)
    y = kernel(xs, ms)
    print(y.shape, y.dtype)



# revision 2
# speedup vs baseline: 1.0915x; 1.0915x over previous
"""Trainium2 Bass kernel v3 for adaptive_high_order_residual_v2 (ORDER=2).

Measured op costs on [128,2752] tiles (HW): ACT pass 2588ns; DVE STT 3030
(any dtype), TS 1700 (2x_2p), TT 16-bit 1586 (2x_1p), TT f32 3000.
GpSimd is ~16ns/elem for wide ops (unusable); TensorTensorReduce crashes
the NRT. So: ACT gets 4 activation passes, DVE gets 2 STT + TS/TT ops.

Per row (m in {0,1}, cnt = sum m, N = 11008):
  T = x*m;  r1 = sum T;  r2 = sum T^2   (exact: invalid entries are 0)
  mean1 = r1/cnt;  var1 = r2/cnt - mean1^2;  s1 = sqrt(var1*2/pi)
  Unmasked ops on T with bias -mean1, corrected for (N-cnt) invalid zeros:
    b1 = Sign(T-mean1): B = B_meas + (N-cnt)*sgn(mean1)
    A_meas = sum|T-mean1|: A = A_meas - (N-cnt)*|mean1|
  mean2 = -s1*B/cnt  (sum q = -s1*sum b1*m)
  var2 = var1 - 2*s1*A/cnt + s1^2 - mean2^2;  s2 = sqrt(var2*2/pi)
  K = mean1+mean2;  C = K - s2
  z' = T - s1*b1  (valid: q - mean2 = z' - K; invalid masked later)
  w2 = 2*s2*(z' > K)  (= s2*b2 + s2)
  out = ((s1*b1 + C) + w2) * m

Engine split per 128x2752 chunk:
  ACT: m16=Copy(m8)+cnt | r2=Square(T) | b1=Sign(T,-mean1)+B | Abs(T,-mean1)+A
  DVE: T=x*m+r1 [STT] | z'=STT(b1,-s1,T)->f16 | w2=TS(z',K,2s2,is_gt,mult)
       in-place | v=TS(b1,s1,C) | vw=TT(v,w2,add) in-place | out=TT(vw,m16)
"""

import sys

import numpy as np

sys.path.insert(0, "/opt/trn_rl_repo")

R = 512          # rows per core
N = 11008        # columns
P = 128          # SBUF partitions per row-block
NBLK = R // P    # 4 blocks per core
CW = 2752        # column chunk width
NCH = N // CW    # 4 chunks per block
NCORES = 8
C2 = 0.6366197723675814  # 2/pi

_CACHE = {}


def _build_program():
    import concourse.bacc as bacc
    import concourse.mybir as mybir
    from concourse.tile import TileContext

    F32 = mybir.dt.float32
    F16 = mybir.dt.float16
    U8 = mybir.dt.uint8
    Alu = mybir.AluOpType
    Act = mybir.ActivationFunctionType

    nc = bacc.Bacc()
    x = nc.dram_tensor("x", [R, N], F32, kind="ExternalInput")
    mk = nc.dram_tensor("mask", [R, N], U8, kind="ExternalInput")
    out = nc.dram_tensor("out", [R, N], F16, kind="ExternalOutput")

    with TileContext(nc) as tc:
        with (
            tc.tile_pool(name="xp", bufs=3) as x_pool,     # x (dead after T)
            tc.tile_pool(name="m8", bufs=3) as m8_pool,
            tc.tile_pool(name="m16", bufs=5) as m16_pool,
            tc.tile_pool(name="tp", bufs=5) as t_pool,     # T (alive to z')
            tc.tile_pool(name="b1", bufs=5) as b1_pool,
            tc.tile_pool(name="av", bufs=3) as av_pool,    # v -> vw in place
            tc.tile_pool(name="zp", bufs=2) as z_pool,     # z' -> w2 in place
            tc.tile_pool(name="ot", bufs=3) as out_pool,
            tc.tile_pool(name="gb", bufs=1) as garb_pool,  # Square/Abs dump
            tc.tile_pool(name="sc", bufs=2) as sc_pool,
        ):
            for b in range(NBLK):
                r0 = b * P

                xt = [
                    x_pool.tile([P, CW], F32, name=f"xt{b}_{c}", tag="xp")
                    for c in range(NCH)
                ]
                mt = [
                    m8_pool.tile([P, CW], U8, name=f"mt{b}_{c}", tag="m8")
                    for c in range(NCH)
                ]
                m16 = [
                    m16_pool.tile([P, CW], F16, name=f"m16_{b}_{c}", tag="m16")
                    for c in range(NCH)
                ]
                tt = [
                    t_pool.tile([P, CW], F32, name=f"tt{b}_{c}", tag="tp")
                    for c in range(NCH)
                ]
                b1t = [
                    b1_pool.tile([P, CW], F16, name=f"b1_{b}_{c}", tag="b1")
                    for c in range(NCH)
                ]
                avt = [
                    av_pool.tile([P, CW], F16, name=f"av{b}_{c}", tag="av")
                    for c in range(NCH)
                ]
                zpt = [
                    z_pool.tile([P, CW], F16, name=f"zp{b}_{c}", tag="zp")
                    for c in range(NCH)
                ]
                ott = [
                    out_pool.tile([P, CW], F16, name=f"ot{b}_{c}", tag="ot")
                    for c in range(NCH)
                ]
                garb = garb_pool.tile([P, CW], F16, name=f"gb{b}", tag="gb")

                # accumulators: chunk-major columns for 2-level tree reduce
                acc1 = sc_pool.tile([P, 3 * NCH], F32, name=f"acc1_{b}", tag="acc1")
                acc2 = sc_pool.tile([P, 2 * NCH], F32, name=f"acc2_{b}", tag="acc2")
                st1 = sc_pool.tile([P, 3], F32, name=f"st1_{b}", tag="st1")
                st2 = sc_pool.tile([P, 2], F32, name=f"st2_{b}", tag="st2")
                red1 = sc_pool.tile([P, 6], F32, name=f"red1_{b}", tag="red1")
                red2 = sc_pool.tile([P, 4], F32, name=f"red2_{b}", tag="red2")
                sv = sc_pool.tile([P, 32], F32, name=f"sv_{b}", tag="sv")

                def col(t, i):
                    return t[:, i : i + 1]

                # ------------- stage 1: load, cnt, r1, r2 -------------------
                for c in range(NCH):
                    nc.sync.dma_start(
                        xt[c][:], x[r0 : r0 + P, c * CW : (c + 1) * CW]
                    )
                    nc.sync.dma_start(
                        mt[c][:], mk[r0 : r0 + P, c * CW : (c + 1) * CW]
                    )
                    # m16 cast + cnt partial
                    nc.scalar.activation(
                        m16[c][:], mt[c][:], Act.Copy, accum_out=col(acc1, 3 * c)
                    )
                    # T = x*m + r1 partial
                    nc.vector.scalar_tensor_tensor(
                        tt[c][:], xt[c][:], 1.0, mt[c][:],
                        Alu.bypass, Alu.mult,
                        accum_out=col(acc1, 3 * c + 1),
                    )
                    # r2 partial = sum T^2 (garbage dst)
                    nc.scalar.activation(
                        garb[:], tt[c][:], Act.Square,
                        accum_out=col(acc1, 3 * c + 2),
                    )

                # tree-reduce acc1 -> st1 = (cnt, r1, r2)
                nc.vector.tensor_add(red1[:, 0:6], acc1[:, 0:6], acc1[:, 6:12])
                nc.vector.tensor_add(st1[:], red1[:, 0:3], red1[:, 3:6])
                cnt, r1, r2 = col(st1, 0), col(st1, 1), col(st1, 2)
                (cm, inv, ninv, mean1, nm1, sg, am, nv, m1sq, e1,
                 var1, v1c, s1, ns1) = (col(sv, i) for i in range(14))
                nc.vector.tensor_scalar(cm, cnt, 1.0, None, Alu.max)
                nc.vector.reciprocal(inv, cm)
                nc.vector.tensor_scalar(ninv, inv, -1.0, None, Alu.mult)
                nc.vector.tensor_mul(mean1, r1, inv)
                nc.vector.tensor_mul(nm1, r1, ninv)
                nc.scalar.activation(sg, mean1, Act.Sign)
                nc.vector.tensor_mul(am, mean1, sg)       # |mean1|
                nc.vector.tensor_scalar(nv, cnt, -1.0, float(N), Alu.mult, Alu.add)
                nc.vector.tensor_mul(m1sq, mean1, mean1)
                nc.vector.tensor_mul(e1, r2, inv)
                nc.vector.tensor_sub(var1, e1, m1sq)
                nc.vector.tensor_scalar(v1c, var1, C2, 1e-30, Alu.mult, Alu.max)
                nc.scalar.activation(s1, v1c, Act.Sqrt)
                nc.vector.tensor_scalar(ns1, s1, -1.0, None, Alu.mult)

                # ------------- stage 2: b1, A ------------------------------
                for c in range(NCH):
                    # b1 = Sign(T - mean1) + B_meas partial
                    nc.scalar.activation(
                        b1t[c][:], tt[c][:], Act.Sign, bias=nm1,
                        accum_out=col(acc2, 2 * c),
                    )
                    # A_meas partial = sum |T - mean1| (garbage dst)
                    nc.scalar.activation(
                        garb[:], tt[c][:], Act.Abs, bias=nm1,
                        accum_out=col(acc2, 2 * c + 1),
                    )

                # tree-reduce acc2 -> st2 = (B_meas, A_meas)
                nc.vector.tensor_add(red2[:, 0:4], acc2[:, 0:4], acc2[:, 4:8])
                nc.vector.tensor_add(st2[:], red2[:, 0:2], red2[:, 2:4])
                Bm, Am = col(st2, 0), col(st2, 1)
                (Bc, Ac, t1, mean2, K, a2, t3, t4, t7, var2, v2c, s2,
                 s2x2, C) = (col(sv, i) for i in range(14, 28))
                nc.vector.tensor_mul(t1, nv, sg)
                nc.vector.tensor_add(Bc, Bm, t1)          # B corrected
                nc.vector.tensor_mul(t1, nv, am)
                nc.vector.tensor_sub(Ac, Am, t1)          # A corrected
                nc.vector.tensor_mul(t1, Bc, ninv)
                nc.vector.tensor_mul(mean2, t1, s1)       # -s1*B/cnt
                nc.vector.tensor_sub(K, mean2, nm1)       # mean1 + mean2
                nc.vector.tensor_mul(t1, Ac, ninv)
                nc.vector.tensor_mul(a2, t1, s1)          # -s1*A/cnt
                nc.vector.tensor_mul(t3, s1, s1)
                nc.vector.tensor_mul(t4, mean2, mean2)
                nc.vector.tensor_add(t7, var1, t3)
                nc.vector.tensor_sub(t7, t7, t4)
                nc.vector.tensor_scalar(t1, a2, 2.0, None, Alu.mult)
                nc.vector.tensor_add(var2, t1, t7)        # t7 + 2*a2
                nc.vector.tensor_scalar(v2c, var2, C2, 1e-30, Alu.mult, Alu.max)
                nc.scalar.activation(s2, v2c, Act.Sqrt)
                nc.vector.tensor_scalar(s2x2, s2, 2.0, None, Alu.mult)
                nc.vector.tensor_sub(C, K, s2)            # K - s2

                # ------------- stage 3: z', w2, v, out ----------------------
                for c in range(NCH):
                    # z' = b1*(-s1) + T  -> f16
                    nc.vector.scalar_tensor_tensor(
                        zpt[c][:], b1t[c][:], ns1, tt[c][:], Alu.mult, Alu.add
                    )
                    # w2 = (z' > K) * 2s2   (in place over z')
                    nc.vector.tensor_scalar(
                        zpt[c][:], zpt[c][:], K, s2x2, Alu.is_gt, Alu.mult
                    )
                    # v = b1*s1 + C
                    nc.vector.tensor_scalar(
                        avt[c][:], b1t[c][:], s1, C, Alu.mult, Alu.add
                    )
                    # vw = v + w2  (in place over v)
                    nc.vector.tensor_add(avt[c][:], avt[c][:], zpt[c][:])
                    # out = vw * m16
                    nc.vector.tensor_mul(ott[c][:], avt[c][:], m16[c][:])
                    nc.sync.dma_start(
                        out[r0 : r0 + P, c * CW : (c + 1) * CW], ott[c][:]
                    )

    nc.finalize()
    return nc


def get_program():
    if "nc" not in _CACHE:
        _CACHE["nc"] = _build_program()
    return _CACHE["nc"]


def kernel(x: np.ndarray, mask: np.ndarray) -> np.ndarray:
    import time

    from concourse.bass_utils import run_bass_kernel_spmd

    x = np.ascontiguousarray(np.asarray(x, dtype=np.float32))
    mask = np.ascontiguousarray(np.asarray(mask))
    if mask.dtype == np.bool_ or mask.dtype == np.uint8:
        mask_u8 = mask.view(np.uint8)
    else:
        mask_u8 = (mask != 0).astype(np.uint8)
    assert x.shape == (R * NCORES, N), x.shape
    assert mask_u8.shape == (R * NCORES, N), mask_u8.shape

    nc = get_program()
    in_maps = [
        {
            "x": x[k * R : (k + 1) * R],
            "mask": mask_u8[k * R : (k + 1) * R],
        }
        for k in range(NCORES)
    ]
    last_err = None
    for attempt in range(3):
        try:
            res = run_bass_kernel_spmd(nc, in_maps, core_ids=list(range(NCORES)))
            return np.concatenate(
                [np.asarray(r["out"]).astype(np.float32) for r in res.results],
                axis=0,
            )
        except Exception as e:  # transient NRT/device hiccups
            last_err = e
            if attempt < 2:
                time.sleep(10)
    raise last_err


if __name__ == "__main__":
    xs = np.random.randn(R * NCORES, N).astype(np.float32)
    ms = (np.random.randint(0, 2, (R * NCORES, N))).astype(bool)
    y = kernel(xs, ms)
    print(y.shape, y.dtype)


# revision 3
# speedup vs baseline: 1.1207x; 1.0268x over previous
"""Trainium2 Bass kernel v3 for adaptive_high_order_residual_v2 (ORDER=2).

Measured op costs on [128,2752] tiles (HW): ACT pass 2588ns; DVE STT 3030
(any dtype), TS 1700 (2x_2p), TT 16-bit 1586 (2x_1p), TT f32 3000.
GpSimd is ~16ns/elem for wide ops (unusable); TensorTensorReduce crashes
the NRT. So: ACT gets 4 activation passes, DVE gets 2 STT + TS/TT ops.

Per row (m in {0,1}, cnt = sum m, N = 11008):
  T = x*m;  r1 = sum T;  r2 = sum T^2   (exact: invalid entries are 0)
  mean1 = r1/cnt;  var1 = r2/cnt - mean1^2;  s1 = sqrt(var1*2/pi)
  Unmasked ops on T with bias -mean1, corrected for (N-cnt) invalid zeros:
    b1 = Sign(T-mean1): B = B_meas + (N-cnt)*sgn(mean1)
    A_meas = sum|T-mean1|: A = A_meas - (N-cnt)*|mean1|
  mean2 = -s1*B/cnt  (sum q = -s1*sum b1*m)
  var2 = var1 - 2*s1*A/cnt + s1^2 - mean2^2;  s2 = sqrt(var2*2/pi)
  K = mean1+mean2;  C = K - s2
  z' = T - s1*b1  (valid: q - mean2 = z' - K; invalid masked later)
  w2 = 2*s2*(z' > K)  (= s2*b2 + s2)
  out = ((s1*b1 + C) + w2) * m

Engine split per 128x2752 chunk:
  ACT: m16=Copy(m8)+cnt | r2=Square(T) | b1=Sign(T,-mean1)+B | Abs(T,-mean1)+A
  DVE: T=x*m+r1 [STT] | z'=STT(b1,-s1,T)->f16 | w2=TS(z',K,2s2,is_gt,mult)
       in-place | v=TS(b1,s1,C) | vw=TT(v,w2,add) in-place | out=TT(vw,m16)
"""

import sys

import numpy as np

sys.path.insert(0, "/opt/trn_rl_repo")

R = 512          # rows per core
N = 11008        # columns
P = 128          # SBUF partitions per row-block
NBLK = R // P    # 4 blocks per core
CW = 2752        # column chunk width
NCH = N // CW    # 4 chunks per block
NCORES = 8
C2 = 0.6366197723675814  # 2/pi

_CACHE = {}


def _build_program():
    import concourse.bacc as bacc
    import concourse.mybir as mybir
    from concourse.tile import TileContext

    F32 = mybir.dt.float32
    F16 = mybir.dt.float16
    U8 = mybir.dt.uint8
    Alu = mybir.AluOpType
    Act = mybir.ActivationFunctionType

    nc = bacc.Bacc()
    x = nc.dram_tensor("x", [R, N], F32, kind="ExternalInput")
    mk = nc.dram_tensor("mask", [R, N], U8, kind="ExternalInput")
    out = nc.dram_tensor("out", [R, N], F16, kind="ExternalOutput")

    with TileContext(nc) as tc:
        with (
            tc.tile_pool(name="xp", bufs=3) as x_pool,     # x (dead after T)
            tc.tile_pool(name="m8", bufs=4) as m8_pool,
            tc.tile_pool(name="m16", bufs=5) as m16_pool,
            tc.tile_pool(name="tp", bufs=6) as t_pool,     # T (alive to z')
            tc.tile_pool(name="b1", bufs=5) as b1_pool,
            tc.tile_pool(name="av", bufs=3) as av_pool,    # v -> vw in place
            tc.tile_pool(name="ot", bufs=3) as out_pool,
            tc.tile_pool(name="gb", bufs=1) as garb_pool,  # Square/Abs dump
            tc.tile_pool(name="sc", bufs=2) as sc_pool,
        ):
            for b in range(NBLK):
                r0 = b * P

                xt = [
                    x_pool.tile([P, CW], F32, name=f"xt{b}_{c}", tag="xp")
                    for c in range(NCH)
                ]
                mt = [
                    m8_pool.tile([P, CW], U8, name=f"mt{b}_{c}", tag="m8")
                    for c in range(NCH)
                ]
                m16 = [
                    m16_pool.tile([P, CW], F16, name=f"m16_{b}_{c}", tag="m16")
                    for c in range(NCH)
                ]
                tt = [
                    t_pool.tile([P, CW], F32, name=f"tt{b}_{c}", tag="tp")
                    for c in range(NCH)
                ]
                b1t = [
                    b1_pool.tile([P, CW], F16, name=f"b1_{b}_{c}", tag="b1")
                    for c in range(NCH)
                ]
                avt = [
                    av_pool.tile([P, CW], F16, name=f"av{b}_{c}", tag="av")
                    for c in range(NCH)
                ]
                ott = [
                    out_pool.tile([P, CW], F16, name=f"ot{b}_{c}", tag="ot")
                    for c in range(NCH)
                ]
                garb = garb_pool.tile([P, CW], F16, name=f"gb{b}", tag="gb")

                # accumulators: chunk-major columns for 2-level tree reduce
                acc1 = sc_pool.tile([P, 3 * NCH], F32, name=f"acc1_{b}", tag="acc1")
                acc2 = sc_pool.tile([P, 2 * NCH], F32, name=f"acc2_{b}", tag="acc2")
                st1 = sc_pool.tile([P, 3], F32, name=f"st1_{b}", tag="st1")
                st2 = sc_pool.tile([P, 2], F32, name=f"st2_{b}", tag="st2")
                red1 = sc_pool.tile([P, 6], F32, name=f"red1_{b}", tag="red1")
                red2 = sc_pool.tile([P, 4], F32, name=f"red2_{b}", tag="red2")
                sv = sc_pool.tile([P, 32], F32, name=f"sv_{b}", tag="sv")

                def col(t, i):
                    return t[:, i : i + 1]

                # ------------- stage 1: load, cnt, r1, r2 -------------------
                for c in range(NCH):
                    nc.sync.dma_start(
                        xt[c][:], x[r0 : r0 + P, c * CW : (c + 1) * CW]
                    )
                    nc.sync.dma_start(
                        mt[c][:], mk[r0 : r0 + P, c * CW : (c + 1) * CW]
                    )
                    # m16 cast + cnt partial
                    nc.scalar.activation(
                        m16[c][:], mt[c][:], Act.Copy, accum_out=col(acc1, 3 * c)
                    )
                    # T = x*m + r1 partial
                    nc.vector.scalar_tensor_tensor(
                        tt[c][:], xt[c][:], 1.0, mt[c][:],
                        Alu.bypass, Alu.mult,
                        accum_out=col(acc1, 3 * c + 1),
                    )
                    # r2 partial = sum T^2 (garbage dst)
                    nc.scalar.activation(
                        garb[:], tt[c][:], Act.Square,
                        accum_out=col(acc1, 3 * c + 2),
                    )

                # tree-reduce acc1 -> st1 = (cnt, r1, r2)
                nc.vector.tensor_add(red1[:, 0:6], acc1[:, 0:6], acc1[:, 6:12])
                nc.vector.tensor_add(st1[:], red1[:, 0:3], red1[:, 3:6])
                cnt, r1, r2 = col(st1, 0), col(st1, 1), col(st1, 2)
                (cm, inv, ninv, mean1, nm1, sg, am, nv, m1sq, e1,
                 var1, v1c, s1, ns1) = (col(sv, i) for i in range(14))
                nc.vector.tensor_scalar(cm, cnt, 1.0, None, Alu.max)
                nc.vector.reciprocal(inv, cm)
                nc.vector.tensor_scalar(ninv, inv, -1.0, None, Alu.mult)
                nc.vector.tensor_mul(mean1, r1, inv)
                nc.vector.tensor_mul(nm1, r1, ninv)
                nc.scalar.activation(sg, mean1, Act.Sign)
                nc.vector.tensor_mul(am, mean1, sg)       # |mean1|
                nc.vector.tensor_scalar(nv, cnt, -1.0, float(N), Alu.mult, Alu.add)
                nc.vector.tensor_mul(m1sq, mean1, mean1)
                nc.vector.tensor_mul(e1, r2, inv)
                nc.vector.tensor_sub(var1, e1, m1sq)
                nc.vector.tensor_scalar(v1c, var1, C2, 1e-30, Alu.mult, Alu.max)
                nc.scalar.activation(s1, v1c, Act.Sqrt)
                nc.vector.tensor_scalar(ns1, s1, -1.0, None, Alu.mult)

                # ------------- stage 2: b1, A ------------------------------
                for c in range(NCH):
                    # b1 = Sign(T - mean1) + B_meas partial
                    nc.scalar.activation(
                        b1t[c][:], tt[c][:], Act.Sign, bias=nm1,
                        accum_out=col(acc2, 2 * c),
                    )
                    # A_meas partial = sum |T - mean1| (garbage dst)
                    nc.scalar.activation(
                        garb[:], tt[c][:], Act.Abs, bias=nm1,
                        accum_out=col(acc2, 2 * c + 1),
                    )

                # tree-reduce acc2 -> st2 = (B_meas, A_meas)
                nc.vector.tensor_add(red2[:, 0:4], acc2[:, 0:4], acc2[:, 4:8])
                nc.vector.tensor_add(st2[:], red2[:, 0:2], red2[:, 2:4])
                Bm, Am = col(st2, 0), col(st2, 1)
                (Bc, Ac, t1, mean2, K, a2, t3, t4, t7, var2, v2c, s2,
                 s2x2, C) = (col(sv, i) for i in range(14, 28))
                nc.vector.tensor_mul(t1, nv, sg)
                nc.vector.tensor_add(Bc, Bm, t1)          # B corrected
                nc.vector.tensor_mul(t1, nv, am)
                nc.vector.tensor_sub(Ac, Am, t1)          # A corrected
                nc.vector.tensor_mul(t1, Bc, ninv)
                nc.vector.tensor_mul(mean2, t1, s1)       # -s1*B/cnt
                nc.vector.tensor_sub(K, mean2, nm1)       # mean1 + mean2
                nc.vector.tensor_mul(t1, Ac, ninv)
                nc.vector.tensor_mul(a2, t1, s1)          # -s1*A/cnt
                nc.vector.tensor_mul(t3, s1, s1)
                nc.vector.tensor_mul(t4, mean2, mean2)
                nc.vector.tensor_add(t7, var1, t3)
                nc.vector.tensor_sub(t7, t7, t4)
                nc.vector.tensor_scalar(t1, a2, 2.0, None, Alu.mult)
                nc.vector.tensor_add(var2, t1, t7)        # t7 + 2*a2
                nc.vector.tensor_scalar(v2c, var2, C2, 1e-30, Alu.mult, Alu.max)
                nc.scalar.activation(s2, v2c, Act.Sqrt)
                nc.vector.tensor_scalar(s2x2, s2, 2.0, None, Alu.mult)
                nc.vector.tensor_sub(C, K, s2)            # K - s2

                # ------------- stage 3: z', w2, v, out ----------------------
                for c in range(NCH):
                    # z' = b1*(-s1) + T  -> f16 (staged in the out tile)
                    nc.vector.scalar_tensor_tensor(
                        ott[c][:], b1t[c][:], ns1, tt[c][:], Alu.mult, Alu.add
                    )
                    # w2 = (z' > K) * 2s2   (in place)
                    nc.vector.tensor_scalar(
                        ott[c][:], ott[c][:], K, s2x2, Alu.is_gt, Alu.mult
                    )
                    # v = b1*s1 + C
                    nc.vector.tensor_scalar(
                        avt[c][:], b1t[c][:], s1, C, Alu.mult, Alu.add
                    )
                    # vw = v + w2  (in place over v)
                    nc.vector.tensor_add(avt[c][:], avt[c][:], ott[c][:])
                    # out = vw * m16 (overwrites the staged w2)
                    nc.vector.tensor_mul(ott[c][:], avt[c][:], m16[c][:])
                    nc.sync.dma_start(
                        out[r0 : r0 + P, c * CW : (c + 1) * CW], ott[c][:]
                    )

    nc.finalize()
    return nc


def get_program():
    if "nc" not in _CACHE:
        _CACHE["nc"] = _build_program()
    return _CACHE["nc"]


def kernel(x: np.ndarray, mask: np.ndarray) -> np.ndarray:
    import time

    from concourse.bass_utils import run_bass_kernel_spmd

    x = np.ascontiguousarray(np.asarray(x, dtype=np.float32))
    mask = np.ascontiguousarray(np.asarray(mask))
    if mask.dtype == np.bool_ or mask.dtype == np.uint8:
        mask_u8 = mask.view(np.uint8)
    else:
        mask_u8 = (mask != 0).astype(np.uint8)
    assert x.shape == (R * NCORES, N), x.shape
    assert mask_u8.shape == (R * NCORES, N), mask_u8.shape

    nc = get_program()
    in_maps = [
        {
            "x": x[k * R : (k + 1) * R],
            "mask": mask_u8[k * R : (k + 1) * R],
        }
        for k in range(NCORES)
    ]
    last_err = None
    for attempt in range(3):
        try:
            res = run_bass_kernel_spmd(nc, in_maps, core_ids=list(range(NCORES)))
            return np.concatenate(
                [np.asarray(r["out"]).astype(np.float32) for r in res.results],
                axis=0,
            )
        except Exception as e:  # transient NRT/device hiccups
            last_err = e
            if attempt < 2:
                time.sleep(10)
    raise last_err


if __name__ == "__main__":
    xs = np.random.randn(R * NCORES, N).astype(np.float32)
    ms = (np.random.randint(0, 2, (R * NCORES, N))).astype(bool)
    y = kernel(xs, ms)
    print(y.shape, y.dtype)


# revision 4
# speedup vs baseline: 1.1340x; 1.0119x over previous
"""Trainium2 Bass kernel v3 for adaptive_high_order_residual_v2 (ORDER=2).

Measured op costs on [128,2752] tiles (HW): ACT pass 2588ns; DVE STT 3030
(any dtype), TS 1700 (2x_2p), TT 16-bit 1586 (2x_1p), TT f32 3000.
GpSimd is ~16ns/elem for wide ops (unusable); TensorTensorReduce crashes
the NRT. So: ACT gets 4 activation passes, DVE gets 2 STT + TS/TT ops.

Per row (m in {0,1}, cnt = sum m, N = 11008):
  T = x*m;  r1 = sum T;  r2 = sum T^2   (exact: invalid entries are 0)
  mean1 = r1/cnt;  var1 = r2/cnt - mean1^2;  s1 = sqrt(var1*2/pi)
  Unmasked ops on T with bias -mean1, corrected for (N-cnt) invalid zeros:
    b1 = Sign(T-mean1): B = B_meas + (N-cnt)*sgn(mean1)
    A_meas = sum|T-mean1|: A = A_meas - (N-cnt)*|mean1|
  mean2 = -s1*B/cnt  (sum q = -s1*sum b1*m)
  var2 = var1 - 2*s1*A/cnt + s1^2 - mean2^2;  s2 = sqrt(var2*2/pi)
  K = mean1+mean2;  C = K - s2
  z' = T - s1*b1  (valid: q - mean2 = z' - K; invalid masked later)
  w2 = 2*s2*(z' > K)  (= s2*b2 + s2)
  out = ((s1*b1 + C) + w2) * m

Engine split per 128x2752 chunk:
  ACT: m16=Copy(m8)+cnt | r2=Square(T) | b1=Sign(T,-mean1)+B | Abs(T,-mean1)+A
  DVE: T=x*m+r1 [STT] | z'=STT(b1,-s1,T)->f16 | w2=TS(z',K,2s2,is_gt,mult)
       in-place | v=TS(b1,s1,C) | vw=TT(v,w2,add) in-place | out=TT(vw,m16)
"""

import sys

import numpy as np

sys.path.insert(0, "/opt/trn_rl_repo")

R = 512          # rows per core
N = 11008        # columns
P = 128          # SBUF partitions per row-block
NBLK = R // P    # 4 blocks per core
CW = 2752        # column chunk width
NCH = N // CW    # 4 chunks per block
NCORES = 8
C2 = 0.6366197723675814  # 2/pi

_CACHE = {}


def _build_program():
    import concourse.bacc as bacc
    import concourse.mybir as mybir
    from concourse.tile import TileContext

    F32 = mybir.dt.float32
    F16 = mybir.dt.float16
    U8 = mybir.dt.uint8
    Alu = mybir.AluOpType
    Act = mybir.ActivationFunctionType

    nc = bacc.Bacc()
    x = nc.dram_tensor("x", [R, N], F32, kind="ExternalInput")
    mk = nc.dram_tensor("mask", [R, N], U8, kind="ExternalInput")
    out = nc.dram_tensor("out", [R, N], F16, kind="ExternalOutput")

    with TileContext(nc) as tc:
        with (
            tc.tile_pool(name="xp", bufs=2) as x_pool,     # x (dead after T)
            tc.tile_pool(name="m8", bufs=4) as m8_pool,
            tc.tile_pool(name="m16", bufs=5) as m16_pool,
            tc.tile_pool(name="tp", bufs=6) as t_pool,     # T (alive to z')
            tc.tile_pool(name="b1", bufs=6) as b1_pool,
            tc.tile_pool(name="av", bufs=3) as av_pool,    # v -> vw in place
            tc.tile_pool(name="ot", bufs=4) as out_pool,
            tc.tile_pool(name="gb", bufs=1) as garb_pool,  # Square/Abs dump
            tc.tile_pool(name="sc", bufs=2) as sc_pool,
        ):
            for b in range(NBLK):
                r0 = b * P

                xt = [
                    x_pool.tile([P, CW], F32, name=f"xt{b}_{c}", tag="xp")
                    for c in range(NCH)
                ]
                mt = [
                    m8_pool.tile([P, CW], U8, name=f"mt{b}_{c}", tag="m8")
                    for c in range(NCH)
                ]
                m16 = [
                    m16_pool.tile([P, CW], F16, name=f"m16_{b}_{c}", tag="m16")
                    for c in range(NCH)
                ]
                tt = [
                    t_pool.tile([P, CW], F32, name=f"tt{b}_{c}", tag="tp")
                    for c in range(NCH)
                ]
                b1t = [
                    b1_pool.tile([P, CW], F16, name=f"b1_{b}_{c}", tag="b1")
                    for c in range(NCH)
                ]
                avt = [
                    av_pool.tile([P, CW], F16, name=f"av{b}_{c}", tag="av")
                    for c in range(NCH)
                ]
                ott = [
                    out_pool.tile([P, CW], F16, name=f"ot{b}_{c}", tag="ot")
                    for c in range(NCH)
                ]
                garb = garb_pool.tile([P, CW], F16, name=f"gb{b}", tag="gb")

                # accumulators: chunk-major columns for 2-level tree reduce
                acc1 = sc_pool.tile([P, 3 * NCH], F32, name=f"acc1_{b}", tag="acc1")
                acc2 = sc_pool.tile([P, 2 * NCH], F32, name=f"acc2_{b}", tag="acc2")
                st1 = sc_pool.tile([P, 3], F32, name=f"st1_{b}", tag="st1")
                st2 = sc_pool.tile([P, 2], F32, name=f"st2_{b}", tag="st2")
                red1 = sc_pool.tile([P, 6], F32, name=f"red1_{b}", tag="red1")
                red2 = sc_pool.tile([P, 4], F32, name=f"red2_{b}", tag="red2")
                sv = sc_pool.tile([P, 32], F32, name=f"sv_{b}", tag="sv")

                def col(t, i):
                    return t[:, i : i + 1]

                # ------------- stage 1: load, cnt, r1, r2 -------------------
                for c in range(NCH):
                    nc.sync.dma_start(
                        xt[c][:], x[r0 : r0 + P, c * CW : (c + 1) * CW]
                    )
                    nc.sync.dma_start(
                        mt[c][:], mk[r0 : r0 + P, c * CW : (c + 1) * CW]
                    )
                    # m16 cast + cnt partial
                    nc.scalar.activation(
                        m16[c][:], mt[c][:], Act.Copy, accum_out=col(acc1, 3 * c)
                    )
                    # T = x*m + r1 partial
                    nc.vector.scalar_tensor_tensor(
                        tt[c][:], xt[c][:], 1.0, mt[c][:],
                        Alu.bypass, Alu.mult,
                        accum_out=col(acc1, 3 * c + 1),
                    )
                    # r2 partial = sum T^2 (garbage dst)
                    nc.scalar.activation(
                        garb[:], tt[c][:], Act.Square,
                        accum_out=col(acc1, 3 * c + 2),
                    )

                # tree-reduce acc1 -> st1 = (cnt, r1, r2)
                nc.vector.tensor_add(red1[:, 0:6], acc1[:, 0:6], acc1[:, 6:12])
                nc.vector.tensor_add(st1[:], red1[:, 0:3], red1[:, 3:6])
                cnt, r1, r2 = col(st1, 0), col(st1, 1), col(st1, 2)
                (cm, inv, ninv, mean1, nm1, sg, am, nv, m1sq, e1,
                 var1, v1c, s1, ns1) = (col(sv, i) for i in range(14))
                nc.vector.tensor_scalar(cm, cnt, 1.0, None, Alu.max)
                nc.vector.reciprocal(inv, cm)
                nc.vector.tensor_scalar(ninv, inv, -1.0, None, Alu.mult)
                nc.vector.tensor_mul(mean1, r1, inv)
                nc.vector.tensor_mul(nm1, r1, ninv)
                nc.scalar.activation(sg, mean1, Act.Sign)
                nc.vector.tensor_mul(am, mean1, sg)       # |mean1|
                nc.vector.tensor_scalar(nv, cnt, -1.0, float(N), Alu.mult, Alu.add)
                nc.vector.tensor_mul(m1sq, mean1, mean1)
                nc.vector.tensor_mul(e1, r2, inv)
                nc.vector.tensor_sub(var1, e1, m1sq)
                nc.vector.tensor_scalar(v1c, var1, C2, 1e-30, Alu.mult, Alu.max)
                nc.scalar.activation(s1, v1c, Act.Sqrt)
                nc.vector.tensor_scalar(ns1, s1, -1.0, None, Alu.mult)

                # ------------- stage 2: b1, A ------------------------------
                for c in range(NCH):
                    # b1 = Sign(T - mean1) + B_meas partial
                    nc.scalar.activation(
                        b1t[c][:], tt[c][:], Act.Sign, bias=nm1,
                        accum_out=col(acc2, 2 * c),
                    )
                    # A_meas partial = sum |T - mean1| (garbage dst)
                    nc.scalar.activation(
                        garb[:], tt[c][:], Act.Abs, bias=nm1,
                        accum_out=col(acc2, 2 * c + 1),
                    )

                # tree-reduce acc2 -> st2 = (B_meas, A_meas)
                nc.vector.tensor_add(red2[:, 0:4], acc2[:, 0:4], acc2[:, 4:8])
                nc.vector.tensor_add(st2[:], red2[:, 0:2], red2[:, 2:4])
                Bm, Am = col(st2, 0), col(st2, 1)
                (Bc, Ac, t1, mean2, K, a2, t3, t4, t7, var2, v2c, s2,
                 s2x2, C) = (col(sv, i) for i in range(14, 28))
                nc.vector.tensor_mul(t1, nv, sg)
                nc.vector.tensor_add(Bc, Bm, t1)          # B corrected
                nc.vector.tensor_mul(t1, nv, am)
                nc.vector.tensor_sub(Ac, Am, t1)          # A corrected
                nc.vector.tensor_mul(t1, Bc, ninv)
                nc.vector.tensor_mul(mean2, t1, s1)       # -s1*B/cnt
                nc.vector.tensor_sub(K, mean2, nm1)       # mean1 + mean2
                nc.vector.tensor_mul(t1, Ac, ninv)
                nc.vector.tensor_mul(a2, t1, s1)          # -s1*A/cnt
                nc.vector.tensor_mul(t3, s1, s1)
                nc.vector.tensor_mul(t4, mean2, mean2)
                nc.vector.tensor_add(t7, var1, t3)
                nc.vector.tensor_sub(t7, t7, t4)
                nc.vector.tensor_scalar(t1, a2, 2.0, None, Alu.mult)
                nc.vector.tensor_add(var2, t1, t7)        # t7 + 2*a2
                nc.vector.tensor_scalar(v2c, var2, C2, 1e-30, Alu.mult, Alu.max)
                nc.scalar.activation(s2, v2c, Act.Sqrt)
                nc.vector.tensor_scalar(s2x2, s2, 2.0, None, Alu.mult)
                nc.vector.tensor_sub(C, K, s2)            # K - s2

                # ------------- stage 3: z', w2, v, out ----------------------
                for c in range(NCH):
                    # z' = b1*(-s1) + T  -> f16 (staged in the out tile)
                    nc.vector.scalar_tensor_tensor(
                        ott[c][:], b1t[c][:], ns1, tt[c][:], Alu.mult, Alu.add
                    )
                    # w2 = (z' > K) * 2s2   (in place)
                    nc.vector.tensor_scalar(
                        ott[c][:], ott[c][:], K, s2x2, Alu.is_gt, Alu.mult
                    )
                    # v = b1*s1 + C  (ACT Identity for the last block to
                    # fill the ACT tail while DVE drains stage 3)
                    if b == NBLK - 1:
                        nc.scalar.activation(
                            avt[c][:], b1t[c][:], Act.Identity,
                            bias=C, scale=s1,
                        )
                    else:
                        nc.vector.tensor_scalar(
                            avt[c][:], b1t[c][:], s1, C, Alu.mult, Alu.add
                        )
                    # vw = v + w2  (in place over v)
                    nc.vector.tensor_add(avt[c][:], avt[c][:], ott[c][:])
                    # out = vw * m16 (overwrites the staged w2)
                    nc.vector.tensor_mul(ott[c][:], avt[c][:], m16[c][:])
                    nc.sync.dma_start(
                        out[r0 : r0 + P, c * CW : (c + 1) * CW], ott[c][:]
                    )

    nc.finalize()
    return nc


def get_program():
    if "nc" not in _CACHE:
        _CACHE["nc"] = _build_program()
    return _CACHE["nc"]


def kernel(x: np.ndarray, mask: np.ndarray) -> np.ndarray:
    import time

    from concourse.bass_utils import run_bass_kernel_spmd

    x = np.ascontiguousarray(np.asarray(x, dtype=np.float32))
    mask = np.ascontiguousarray(np.asarray(mask))
    if mask.dtype == np.bool_ or mask.dtype == np.uint8:
        mask_u8 = mask.view(np.uint8)
    else:
        mask_u8 = (mask != 0).astype(np.uint8)
    assert x.shape == (R * NCORES, N), x.shape
    assert mask_u8.shape == (R * NCORES, N), mask_u8.shape

    nc = get_program()
    in_maps = [
        {
            "x": x[k * R : (k + 1) * R],
            "mask": mask_u8[k * R : (k + 1) * R],
        }
        for k in range(NCORES)
    ]
    last_err = None
    for attempt in range(3):
        try:
            res = run_bass_kernel_spmd(nc, in_maps, core_ids=list(range(NCORES)))
            return np.concatenate(
                [np.asarray(r["out"]).astype(np.float32) for r in res.results],
                axis=0,
            )
        except Exception as e:  # transient NRT/device hiccups
            last_err = e
            if attempt < 2:
                time.sleep(10)
    raise last_err


if __name__ == "__main__":
    xs = np.random.randn(R * NCORES, N).astype(np.float32)
    ms = (np.random.randint(0, 2, (R * NCORES, N))).astype(bool)
    y = kernel(xs, ms)
    print(y.shape, y.dtype)
